# revision 5
# baseline (speedup 1.0000x reference)
"""Trainium2 Bass kernel: 7x7 single-channel 2D convolution (zero-padded),
data-parallel over 8 NeuronCores (8 images per core).

Decomposition (per image, H=W=512, k=7, pad=3):
  out[h, w] = sum_{dy,dx} k[dy,dx] * Xp[h+dy, w+dx]      (Xp = zero-padded X)

The H-direction convolution for one kernel column dx is a banded matmul:
  C_dx[h, j] = sum_r A_dx[r, h] * Xp[r, j],   A_dx[r, h] = k[r-h, dx]
and the W-direction shift-accumulate over dx is realized by feeding the
moving operand a column-shifted slice Xp[:, dx:dx+512] while accumulating
all 7 dx into the same PSUM region (PSUM has_written accumulation).

H is tiled into 20 windows of 32 input rows (stride 26 -> 26 outputs per
window). Each window's 7 matmuls are K=32, M=26, N=512 and run on one of
the PE's 16 independent 32x32 sub-tiles (4 SBUF row-strips x 4 PSUM
col-strips), so up to 16 windows execute concurrently on the array.
4 windows (4 distinct PSUM col-strips) share one PSUM bank; the bank is
evacuated with a single [128, 512] copy alternating DVE/ACT, then DMA'd
to HBM.
"""

import numpy as np

B = 64          # total images
NC = 8          # neuron cores
BPC = B // NC   # images per core
H = W = 512
KS = 7
PAD = KS // 2
WIN = 32        # window input rows (K of each matmul)
OUT_W = 26      # outputs per window (WIN - KS + 1)
NWIN = 20       # windows per image (ceil(512/26))
NQ = 5          # q blocks per image (NWIN / 4)
PADW = W + 2 * PAD           # 518 padded cols
PADH_VIEW = 26 * 3 + 104 * 5  # 598 rows: enough for the strided view span
F32 = np.float32


def _host_prep(X, kern):
    """Returns per-core padded image arrays and the band-matrix tensor."""
    xs = []
    for c in range(NC):
        xp = np.zeros((BPC, PADH_VIEW, PADW), dtype=F32)
        xp[:, PAD:PAD + H, PAD:PAD + W] = X[c * BPC:(c + 1) * BPC, 0]
        xs.append(xp)
    # bands[32*i + r, dx, m] = kern[r - m, dx] for 0 <= r - m < 7, m < 26
    bands = np.zeros((128, KS, 32), dtype=F32)
    for r in range(WIN):
        for m in range(OUT_W):
            dy = r - m
            if 0 <= dy < KS:
                bands[r, :, m] = kern[dy, :]
    for i in range(1, 4):
        bands[32 * i:32 * i + 32] = bands[0:32]
    return xs, bands


def build_bass():
    from concourse import bass, mybir
    from concourse import tile

    dt = mybir.dt.float32
    nc = bass.Bass("TRN2", target_bir_lowering=False, debug=False)

    xpad_d = nc.dram_tensor("xpad", [BPC, PADH_VIEW, PADW], dt, kind="ExternalInput")
    bands_d = nc.dram_tensor("bands", [128, KS, 32], dt, kind="ExternalInput")
    y_d = nc.dram_tensor("y", [BPC, H, W], dt, kind="ExternalOutput")

    with tile.TileContext(nc) as tc:
        with (
            tc.tile_pool(name="const", bufs=1) as const_pool,
            tc.tile_pool(name="win", bufs=3) as win_pool,
            tc.tile_pool(name="ps", bufs=8, space=bass.MemorySpace.PSUM) as psum_pool,
            tc.tile_pool(name="st", bufs=6) as stage_pool,
        ):
            bands_sb = const_pool.tile([128, KS, 32], dt, name="bands_sb")
            nc.sync.dma_start(out=bands_sb[:], in_=bands_d[:])

            for b in range(BPC):
                win = win_pool.tile([128, NQ, PADW], dt, name="win", tag="win")
                for i in range(4):
                    src = (
                        xpad_d[b, 26 * i:26 * i + 520, :]
                        .rearrange("(q r) c -> q r c", r=104)[:, 0:WIN, :]
                        .rearrange("q r c -> r q c")
                    )
                    nc.sync.dma_start(out=win[32 * i:32 * i + 32, :, :], in_=src)

                for q in range(NQ):
                    grp = b * NQ + q
                    psumt = psum_pool.tile([128, W], dt, name="ps", tag="ps")
                    for dx in range(KS):
                        for l in range(4):
                            j = (l + grp) % 4
                            nc.tensor.matmul(
                                psumt[32 * j:32 * j + OUT_W, :],
                                bands_sb[32 * l:32 * l + 32, dx, 0:OUT_W],
                                win[32 * l:32 * l + 32, q, dx:dx + W],
                                start=(dx == 0),
                                stop=(dx == KS - 1),
                                tile_position=(32 * l, 32 * j),
                            )
                    stage = stage_pool.tile([128, W], dt, name="st", tag="st")
                    if grp % 2 == 0:
                        nc.vector.tensor_copy(stage[:], psumt[:])
                    else:
                        nc.scalar.copy(stage[:], psumt[:])
                    for l in range(4):
                        j = (l + grp) % 4
                        r0 = 104 * q + 26 * l
                        nrows = min(OUT_W, H - r0)
                        nc.gpsimd.dma_start(
                            out=y_d[b, r0:r0 + nrows, :],
                            in_=stage[32 * j:32 * j + nrows, :],
                        )
    _split_multi_waits(nc, mybir)
    return nc


def _split_multi_waits(nc, mybir):
    """This walrus build accepts at most one semaphore wait per
    instruction; Tile can emit several. Hoist all but the last wait onto
    NoOps inserted just before, on the same engine queue (engine programs
    preserve relative instruction order, so the waits still gate the
    original instruction)."""
    uid = 0
    for fn in nc.m.functions:
        for blk in fn.blocks:
            insts = blk.instructions
            out = []
            for ins in insts:
                si = getattr(ins, "sync_info", None)
                if si is not None and len(si.on_wait) > 1:
                    waits = list(si.on_wait)
                    for w in waits[:-1]:
                        nop = mybir.InstNoOp(
                            name=f"waitnop_{uid}", engine=ins.engine
                        )
                        nop.sync_info = mybir.SyncInfo(on_wait=[w], on_update=[])
                        out.append(nop)
                        uid += 1
                    ins.sync_info = mybir.SyncInfo(
                        on_wait=[waits[-1]], on_update=list(si.on_update)
                    )
                out.append(ins)
            blk.instructions = out


_CACHED = {}


def kernel(X, kernel):
    X = np.ascontiguousarray(np.asarray(X), dtype=F32)
    kern = np.asarray(kernel, dtype=F32)
    assert X.shape == (B, 1, H, W), X.shape
    assert kern.shape == (KS, KS), kern.shape

    from concourse.bass_utils import run_bass_kernel_spmd

    if "nc" not in _CACHED:
        _CACHED["nc"] = build_bass()
    nc = _CACHED["nc"]

    xs, bands = _host_prep(X, kern)
    in_maps = [{"xpad": xs[c], "bands": bands} for c in range(NC)]
    res = run_bass_kernel_spmd(nc, in_maps, list(range(NC)))
    out = np.empty((B, 1, H, W), dtype=F32)
    for c in range(NC):
        out[c * BPC:(c + 1) * BPC, 0] = res.results[c]["y"]
    return out
